# revision 1
# baseline (speedup 1.0000x reference)
"""TAGConv(K=3, in=1, out=128) + gcn_norm + MLP head, sharded over 8 trn2 cores.

Math (identical to reference.py):
  deg[c] = segsum_dest(edge_attr);  dis = where(deg>0, rsqrt(max(deg,1e-30)), 0)
  v_0 = dis * x;  per hop: h_k = dis * segsum_dest(ea * v_{k-1}[row]),
  v_k = dis * h_k   (gcn_norm folded into the gathered value v = dis*h)
  out = relu([x,h1,h2,h3] @ W4 + bias); z = relu(out@w1+b1); y = relu(z@w2+b2)

Device mapping:
 - destinations sharded: core owns D=63488 padded dests as [128 partition, T=496]
 - edges stored dest-major, padded to B=64 slots per dest
 - random source gather: GPSIMD IndirectCopy from a 16-slice replicated SBUF
   table (partition 16c+j holds table slice j); each slot yields 16 candidate
   values, combined with a host-precomputed masked-ea array via a block-sum
   matmul + diagonal mask, then reduced over slots.
 - hop tables rebuilt with an HBM AllGather collective between hops.
"""

import numpy as np

N = 500000
E = 16000000
NC = 8
NSH = N // NC          # 62500 real nodes per core
T = 496                # dest tiles per core
GB = 8                 # tiles per iteration in the deg pass
D = 128 * T            # padded nodes per core = 63488
NTAB = NC * D          # padded global table size = 507904
SLICE = NTAB // 16     # table slice per partition group lane = 31744
DIM = 128
B = 64                 # slot budget per dest (asserted against real max deg)


def _host_prep(x, edge_index, edge_attr):
    """Build per-core slot arrays. Pure layout/sharding work.

    Destinations are placed into (partition, tile) positions by descending
    degree rank, so each tile's per-dest slot budget B_t tracks the actual
    degrees in that tile instead of the global max.
    """
    row = np.asarray(edge_index[0], dtype=np.int64)
    col = np.asarray(edge_index[1], dtype=np.int64)
    ea = np.asarray(edge_attr, dtype=np.float32)
    x = np.asarray(x, dtype=np.float32).reshape(-1)

    core = col // NSH
    l_loc = col % NSH

    # per-core degree of every real dest
    cnt_all = np.bincount(core * NSH + l_loc, minlength=NC * NSH)
    cnt_all = cnt_all.reshape(NC, NSH)

    # per-core placement: rank r (descending degree) -> position (p=r%128, t=r//128)
    perm = np.argsort(-cnt_all, axis=1, kind="stable")   # rank -> orig l
    inv = np.empty_like(perm)
    ar = np.arange(NSH)
    for c in range(NC):
        inv[c, perm[c]] = ar                              # orig l -> rank

    # per-tile budget: max degree among the tile's dests across all cores
    deg_rank = np.take_along_axis(cnt_all, perm, axis=1)  # [NC, NSH] descending
    deg_rank_pad = np.zeros((NC, D), np.int64)
    deg_rank_pad[:, :NSH] = deg_rank
    per_tile = deg_rank_pad.reshape(NC, T, 128)           # rank r=(t*128+p)
    Bt = per_tile.max(axis=(0, 2))                        # [T]
    Bt = np.maximum(((Bt + 3) // 4) * 4, 4).astype(np.int64)
    assert Bt.max() <= 64

    # pack consecutive tiles into IndirectCopy call groups (16*BG <= 1024)
    groups = []           # (t_start, n_tiles, BG)
    t0 = 0
    while t0 < T:
        bg = 0; n = 0
        while t0 + n < T and 16 * (bg + Bt[t0 + n]) <= 1024:
            bg += int(Bt[t0 + n]); n += 1
        groups.append((t0, n, bg))
        t0 += n
    NG = len(groups)
    grp_of_t = np.zeros(T, np.int64)
    cum_in_grp = np.zeros(T, np.int64)
    BG_of_t = np.zeros(T, np.int64)
    for g, (ts, n, bg) in enumerate(groups):
        c = 0
        for k in range(n):
            grp_of_t[ts + k] = g
            cum_in_grp[ts + k] = c
            BG_of_t[ts + k] = bg
            c += int(Bt[ts + k])
    goff_base = np.concatenate([[0], np.cumsum([128 * bg for _, _, bg in groups])])
    gmea_base = np.concatenate([[0], np.cumsum([128 * 16 * bg for _, _, bg in groups])])
    OFFSZ = int(goff_base[-1]); MEASZ = int(gmea_base[-1])

    # rank/position of every edge's dest
    rank_e = inv[core, l_loc]                             # [E]
    t_e = rank_e // 128
    p_e = rank_e % 128

    # order edges by (core, rank) and get within-dest rank
    order = np.lexsort((rank_e, core))
    core_s = core[order]; t_s = t_e[order]; p_s = p_e[order]
    rank_s = rank_e[order]
    ea_s = ea[order]
    row_s = row[order]

    key = core_s * NSH + rank_s
    uniq, start, cnt = np.unique(key, return_index=True, return_counts=True)
    lane = np.arange(E, dtype=np.int64) - np.repeat(start, cnt)

    # source table position: device DRAM layout is partition-major, so rank r
    # lives at DRAM position (r % 128) * T + (r // 128)
    rsrc = inv[row_s // NSH, row_s % NSH]
    gsrc_s = (row_s // NSH) * D + (rsrc % 128) * T + (rsrc // 128)
    sl_s = gsrc_s // SLICE
    off_s = gsrc_s % SLICE

    # deg-pass array (fixed B=64 padded layout on ranked positions)
    pos_deg = ((t_s // GB) * (128 * GB * B) + p_s * (GB * B)
               + (t_s % GB) * B + lane)
    SLOTS = D * B
    ea_flat = np.zeros(NC * SLOTS, dtype=np.float32)
    ea_flat[core_s * SLOTS + pos_deg] = ea_s
    ea_ar = ea_flat.reshape(NC, SLOTS)

    # gather-pass arrays, group-major layout
    g_e = grp_of_t[t_s]
    BG_e = BG_of_t[t_s]
    w_e = cum_in_grp[t_s] + lane                     # lane within group
    pos_off = goff_base[g_e] + p_s * BG_e + w_e
    off_flat = np.zeros(NC * OFFSZ, dtype=np.uint16)
    off_flat[core_s * OFFSZ + pos_off] = off_s.astype(np.uint16)
    off_ar = off_flat.reshape(NC, OFFSZ)

    grp = (p_s // 16) * 16
    jp = p_s % 16
    pos_mea = (gmea_base[g_e] + (grp + sl_s) * (16 * BG_e) + w_e * 16 + jp)
    mea_flat = np.zeros(NC * MEASZ, dtype=np.float32)
    mea_flat[core_s * MEASZ + pos_mea] = ea_s
    mea_ar = mea_flat.reshape(NC, MEASZ)

    # x in ranked table layout (partition-major device positions)
    xfull = np.zeros(NTAB, dtype=np.float32)
    n = np.arange(N, dtype=np.int64)
    rn = inv[n // NSH, n % NSH]
    xfull[(n // NSH) * D + (rn % 128) * T + (rn // 128)] = x

    dm = np.zeros((128, 16 * B), dtype=np.float32)
    for pp_ in range(128):
        dm[pp_, np.arange(B) * 16 + (pp_ % 16)] = 1.0
    R = np.zeros((128, 128), dtype=np.float32)
    for q in range(128):
        R[q, (q // 16) * 16:(q // 16) * 16 + 16] = 1.0
    return ea_ar, off_ar, mea_ar, xfull, dm, R, Bt, groups, inv


def _build(bass, tile, mybir, Bt, groups, debug=False):
    import concourse.bacc as bacc
    dt = mybir.dt
    fp = dt.float32
    SLOTS = D * B
    W16 = 16 * B                       # 1024: IndirectCopy out free size
    assert W16 <= 1024

    nc = bacc.Bacc("TRN2", num_devices=NC)
    Bt = [int(b) for b in Bt]
    goff_base = np.concatenate(
        [[0], np.cumsum([128 * bg for _, _, bg in groups])]).astype(int)
    gmea_base = np.concatenate(
        [[0], np.cumsum([128 * 16 * bg for _, _, bg in groups])]).astype(int)
    OFFSZ = int(goff_base[-1])
    MEASZ = int(gmea_base[-1])
    ea_h = nc.dram_tensor("ea", [SLOTS], fp, kind="ExternalInput")
    off_h = nc.dram_tensor("off", [OFFSZ], dt.uint16, kind="ExternalInput")
    mea_h = nc.dram_tensor("mea", [MEASZ], fp, kind="ExternalInput")
    xf_h = nc.dram_tensor("xfull", [NTAB], fp, kind="ExternalInput")
    xs_h = nc.dram_tensor("xsh", [D], fp, kind="ExternalInput")
    dm_h = nc.dram_tensor("dmask", [128, W16], fp, kind="ExternalInput")
    r_h = nc.dram_tensor("rmat", [128, 128], fp, kind="ExternalInput")
    w4_h = nc.dram_tensor("w4", [4, DIM], fp, kind="ExternalInput")
    bc_h = nc.dram_tensor("biasc", [DIM, 1], fp, kind="ExternalInput")
    w1_h = nc.dram_tensor("w1", [DIM, DIM], fp, kind="ExternalInput")
    b1_h = nc.dram_tensor("b1c", [DIM, 1], fp, kind="ExternalInput")
    w2_h = nc.dram_tensor("w2", [DIM, 1], fp, kind="ExternalInput")
    b2_h = nc.dram_tensor("b2c", [1, 1], fp, kind="ExternalInput")
    y_h = nc.dram_tensor("y", [D], fp, kind="ExternalOutput")
    if debug:
        dbg_h = {n: nc.dram_tensor(n, [D], fp, kind="ExternalOutput")
                 for n in ("degO", "disO", "h0O", "h1O", "h2O")}

    NIT = T // GB
    WD = GB * B                        # deg-pass free size per iteration

    # Pin the gather table at a fixed SBUF offset: the IndirectCopy ucode's
    # batched tensor-reads can touch [base - (SLICE-1)*2, base + 2*(SLICE-1)*2]
    # bytes, so the table must sit away from both SBUF ends.  bf16 keeps the
    # whole span inside the 224KB partition.
    TAB_OFF = 65536
    arena_bytes = TAB_OFF + SLICE * 2 - nc.sbuf_base
    arena_ctx = nc.sbuf_tensor([128, arena_bytes], dt.uint8)
    arena = arena_ctx.__enter__()
    tabh = nc.alloc_sbuf_tensor_at(
        "tabt", [128, SLICE], dt.bfloat16, offset=TAB_OFF)

    with tile.TileContext(nc) as tc:
        with (
            tc.tile_pool(name="pers", bufs=1) as pers,
            tc.tile_pool(name="dram", bufs=1, space="DRAM") as dram,
        ):
            ea_v = ea_h[:].rearrange("(i p w) -> i p w", p=128, w=WD)
            xf_v = xf_h[:].rearrange("(p f) -> p f", p=128)

            deg = pers.tile([128, T], fp)
            dis = pers.tile([128, T], fp)
            rs = pers.tile([128, 128], fp)
            dms = pers.tile([128, W16], fp)
            nc.sync.dma_start(rs[:], r_h[:])
            nc.sync.dma_start(dms[:], dm_h[:])

            # ---------- pass 0: deg -> dis ----------
            with tc.tile_pool(name="dpool", bufs=3) as dpool:
                for i in range(NIT):
                    eat = dpool.tile([128, WD], fp, tag="eat")
                    nc.sync.dma_start(eat[:], ea_v[i])
                    nc.vector.reduce_sum(
                        deg[:, i * GB:(i + 1) * GB],
                        eat[:].rearrange("p (g b) -> p g b", b=B),
                        axis=mybir.AxisListType.X,
                    )
                scr1 = dpool.tile([128, T], fp, tag="scr1")
                scr2 = dpool.tile([128, T], fp, tag="scr2")
                nc.vector.tensor_single_scalar(
                    scr1[:], deg[:], 0.0, mybir.AluOpType.is_gt)
                nc.vector.tensor_scalar_max(scr2[:], deg[:], 1e-30)
                nc.vector.reciprocal(scr2[:], scr2[:])
                nc.vector.tensor_mul(scr2[:], scr2[:], scr1[:])
                nc.scalar.activation(
                    dis[:], scr2[:], mybir.ActivationFunctionType.Sqrt)
                if debug:
                    nc.sync.dma_start(
                        dbg_h["degO"][:].rearrange("(p t) -> p t", p=128), deg[:])
                    nc.sync.dma_start(
                        dbg_h["disO"][:].rearrange("(p t) -> p t", p=128), dis[:])

                # dis allgather + v0 table
                dis_d = dram.tile([D], fp, tag="dis_d")
                disf_d = dram.tile([NTAB], fp, tag="disf")
                nc.sync.dma_start(
                    dis_d[:].rearrange("(p t) -> p t", p=128), dis[:])
                nc.gpsimd.collective_compute(
                    "AllGather", mybir.AluOpType.bypass,
                    replica_groups=[list(range(NC))],
                    ins=[dis_d[:].opt()], outs=[disf_d[:].opt()],
                )
                vt = [dram.tile([NTAB], dt.bfloat16, tag=f"vt{k}",
                                name=f"vt{k}") for k in range(3)]
                disf_v = disf_d[:].rearrange("(p f) -> p f", p=128)
                vt0_v = vt[0][:].rearrange("(p f) -> p f", p=128)
                FV = NTAB // 128 // 4
                for i in range(4):
                    sl = slice(i * FV, (i + 1) * FV)
                    xt = dpool.tile([128, FV], fp, tag="xt")
                    dft = dpool.tile([128, FV], fp, tag="dft")
                    xtb = dpool.tile([128, FV], dt.bfloat16, tag="xtb")
                    nc.sync.dma_start(xt[:], xf_v[:, sl])
                    nc.sync.dma_start(dft[:], disf_v[:, sl])
                    nc.vector.tensor_mul(xt[:], xt[:], dft[:])
                    nc.vector.tensor_copy(xtb[:], xt[:])
                    nc.sync.dma_start(vt0_v[:, sl], xtb[:])

            # ---------- hops ----------
            hk_d = [dram.tile([D], fp, tag=f"h{k}d", name=f"h{k}d")
                    for k in range(3)]
            hsh = pers.tile([128, T], fp, tag="hsh")
            vsh = pers.tile([128, T], fp, tag="vsh")
            with (
                tc.tile_pool(name="hpool", bufs=3) as hp,
                tc.tile_pool(name="hpsum", bufs=2, space="PSUM") as pp,
            ):
                for k in range(3):
                    # load replicated table: partition 16c+j <- slice j
                    vt_s = vt[k][:].rearrange("(j f) -> j f", j=16)
                    for c in range(8):
                        nc.sync.dma_start(tabh[16 * c:16 * c + 16, :], vt_s)
                    for g, (ts, ntl, bg) in enumerate(groups):
                        wt = 16 * bg
                        half = wt // 2
                        offt = hp.tile([128, bg], dt.uint16, tag="offt")
                        gt = hp.tile([128, wt], dt.bfloat16, tag="gt")
                        meat = hp.tile([128, wt], fp, tag="meat")
                        prod = hp.tile([128, wt], fp, tag="prod")
                        nc.sync.dma_start(
                            offt[:],
                            off_h[int(goff_base[g]):int(goff_base[g + 1])]
                            .rearrange("(p w) -> p w", p=128))
                        nc.sync.dma_start(
                            meat[:],
                            mea_h[int(gmea_base[g]):int(gmea_base[g + 1])]
                            .rearrange("(p w) -> p w", p=128))
                        nc.gpsimd.indirect_copy(gt[:], tabh[:, :], offt[:], True)
                        nc.vector.tensor_mul(prod[:], gt[:], meat[:])
                        ps1 = pp.tile([128, 512], fp, tag="ps1")
                        ps2 = pp.tile([128, 512], fp, tag="ps2")
                        nc.tensor.matmul(ps1[:, :half], rs[:], prod[:, :half],
                                         start=True, stop=True)
                        nc.tensor.matmul(ps2[:, :half], rs[:], prod[:, half:],
                                         start=True, stop=True)
                        nc.vector.tensor_mul(prod[:, :half], ps1[:, :half],
                                             dms[:, :half])
                        nc.vector.tensor_mul(prod[:, half:], ps2[:, :half],
                                             dms[:, half:wt])
                        cum = 0
                        for kk in range(ntl):
                            bt = Bt[ts + kk]
                            nc.vector.reduce_sum(
                                hsh[:, ts + kk:ts + kk + 1],
                                prod[:, cum * 16:(cum + bt) * 16].rearrange(
                                    "p (o w j) -> p o w j", j=16, o=1),
                                axis=mybir.AxisListType.XY,
                            )
                            cum += bt
                    nc.vector.tensor_mul(hsh[:], hsh[:], dis[:])
                    nc.sync.dma_start(
                        hk_d[k][:].rearrange("(p t) -> p t", p=128), hsh[:])
                    if debug:
                        nc.sync.dma_start(
                            dbg_h[f"h{k}O"][:].rearrange("(p t) -> p t", p=128),
                            hsh[:])
                    if k < 2:
                        vd = dram.tile([D], dt.bfloat16, tag=f"v{k}d",
                                       name=f"v{k}d")
                        vshb = pers.tile([128, T], dt.bfloat16, tag="vshb")
                        nc.vector.tensor_mul(vsh[:], hsh[:], dis[:])
                        nc.vector.tensor_copy(vshb[:], vsh[:])
                        nc.sync.dma_start(
                            vd[:].rearrange("(p t) -> p t", p=128), vshb[:])
                        nc.gpsimd.collective_compute(
                            "AllGather", mybir.AluOpType.bypass,
                            replica_groups=[list(range(NC))],
                            ins=[vd[:].opt()], outs=[vt[k + 1][:].opt()],
                        )

            # ---------- dense tail ----------
            with (
                tc.tile_pool(name="tpool", bufs=2) as tp,
                tc.tile_pool(name="tpsum", bufs=2, space="PSUM") as pp,
            ):
                w4s = pers.tile([4, DIM], fp)
                bcs = pers.tile([DIM, 1], fp)
                w1s = pers.tile([DIM, DIM], fp)
                b1s = pers.tile([DIM, 1], fp)
                w2s = pers.tile([DIM, 1], fp)
                b2s = pers.tile([1, 1], fp)
                nc.sync.dma_start(w4s[:], w4_h[:])
                nc.sync.dma_start(bcs[:], bc_h[:])
                nc.sync.dma_start(w1s[:], w1_h[:])
                nc.sync.dma_start(b1s[:], b1_h[:])
                nc.sync.dma_start(w2s[:], w2_h[:])
                nc.sync.dma_start(b2s[:], b2_h[:])

                FB = 1984
                NTC = D // FB
                NMM = FB // 496
                for ci in range(NTC):
                    h4 = tp.tile([4, FB], fp, tag="h4")
                    sl = slice(ci * FB, (ci + 1) * FB)
                    nc.sync.dma_start(
                        h4[0:1, :], xs_h[sl].rearrange("(o f) -> o f", o=1))
                    for k in range(3):
                        nc.sync.dma_start(
                            h4[k + 1:k + 2, :],
                            hk_d[k][sl].rearrange("(o f) -> o f", o=1))
                    yrow = tp.tile([1, FB], fp, tag="yrow")
                    for j in range(NMM):
                        js = slice(j * 496, (j + 1) * 496)
                        ps1 = pp.tile([DIM, 496], fp, tag="tps1")
                        ps2 = pp.tile([DIM, 496], fp, tag="tps2")
                        ps3 = pp.tile([1, 496], fp, tag="tps3")
                        o1 = tp.tile([DIM, 496], fp, tag="o1")
                        z1 = tp.tile([DIM, 496], fp, tag="z1")
                        nc.tensor.matmul(ps1[:], w4s[:], h4[:, js],
                                         start=True, stop=True)
                        nc.scalar.activation(
                            o1[:], ps1[:], mybir.ActivationFunctionType.Relu,
                            bias=bcs[:, 0:1])
                        nc.tensor.matmul(ps2[:], w1s[:], o1[:],
                                         start=True, stop=True)
                        nc.scalar.activation(
                            z1[:], ps2[:], mybir.ActivationFunctionType.Relu,
                            bias=b1s[:, 0:1])
                        nc.tensor.matmul(ps3[:], w2s[:], z1[:],
                                         start=True, stop=True)
                        nc.scalar.activation(
                            yrow[:, js], ps3[:],
                            mybir.ActivationFunctionType.Relu,
                            bias=b2s[:, 0:1])
                    nc.sync.dma_start(
                        y_h[sl].rearrange("(o f) -> o f", o=1), yrow[:])
    arena_ctx.__exit__(None, None, None)
    nc.compile()
    return nc


_CACHE = {}


def kernel(x, edge_index, edge_attr, W0, W1, W2, W3, bias,
           mlp_w1, mlp_b1, mlp_w2, mlp_b2):
    import os
    import concourse.bass as bass
    import concourse.tile as tile
    import concourse.mybir as mybir
    from concourse.bass_utils import run_bass_kernel_spmd

    ea_ar, off_ar, mea_ar, xfull, dm, R, Bt, groups, inv = _host_prep(
        x, edge_index, edge_attr)

    key = tuple(Bt.tolist())
    if key not in _CACHE:
        _CACHE.clear()
        _CACHE[key] = _build(bass, tile, mybir, Bt, groups)
    nc = _CACHE[key]

    w4 = np.concatenate([np.asarray(w, np.float32).reshape(1, DIM)
                         for w in (W0, W1, W2, W3)], axis=0)
    common = {
        "xfull": xfull, "dmask": dm, "rmat": R,
        "w4": np.ascontiguousarray(w4),
        "biasc": np.asarray(bias, np.float32).reshape(DIM, 1),
        "w1": np.ascontiguousarray(np.asarray(mlp_w1, np.float32)),
        "b1c": np.asarray(mlp_b1, np.float32).reshape(DIM, 1),
        "w2": np.ascontiguousarray(
            np.asarray(mlp_w2, np.float32).reshape(DIM, 1)),
        "b2c": np.asarray(mlp_b2, np.float32).reshape(1, 1),
    }
    in_maps = []
    for c in range(NC):
        m = dict(common)
        m["ea"] = ea_ar[c]
        m["off"] = off_ar[c]
        m["mea"] = mea_ar[c]
        m["xsh"] = xfull[c * D:(c + 1) * D]
        in_maps.append(m)

    # The axon terminal pool occasionally has a wedged device left over from
    # an earlier crashed run; execution then fails with
    # NRT_EXEC_UNIT_UNRECOVERABLE / worker-hung-up.  Retry a couple of times
    # before giving up.
    import time as _time
    last_exc = None
    for attempt in range(3):
        try:
            res = run_bass_kernel_spmd(nc, in_maps, core_ids=list(range(NC)),
                                       trace=bool(os.environ.get("KTRACE")))
            break
        except Exception as e:  # noqa: BLE001
            last_exc = e
            _time.sleep(5.0)
    else:
        raise last_exc
    globals()["LAST_RESULTS"] = res
    y = np.concatenate(
        [res.results[c]["y"][(inv[c, :NSH] % 128) * T + (inv[c, :NSH] // 128)]
         for c in range(NC)])
    return y.reshape(N, 1).astype(np.float32)

